# revision 59
# baseline (speedup 1.0000x reference)
"""Trainium2 Bass kernel v2 for multi-head attention with RoPE (causal).

Contract: kernel(**inputs) takes FULL unsharded inputs
  x (B,T,C) f32, w_qkv (3C,C), b_qkv (3C,), w_out (C,C), b_out (C,)
and returns the FULL (B,T,C) f32 output.

Sharding: heads split across 8 NeuronCores (tensor parallel, 2 heads per
core). Each core computes its heads' attention and a partial output
projection over its 256 columns of att_v; the host sums the 8 partials
(the "all-reduce") and adds the bias constant.

v2 changes vs v1:
- all matmul operands bf16 (halves DMA + SBUF, full PE rate at any tile
  size); PSUM accumulation stays fp32
- softmax denominator via vector-accumulated e_sum + ONE ones-matmul per
  (head, chunk) instead of 320 PE ones-matmuls; 1/L via scalar Ln/Exp and
  a broadcast matmul (gpsimd partition_all_reduce / vector reciprocal
  measured 3.4-3.6us each — far too slow for the critical path)
- output projection interleaved into the attention loop as PE gap filler
- k/v SBUF double-buffered across batches; deeper x-tile prefetch
- output partials written bf16 (halves the write DMA)
"""

import sys, math, os, json, tempfile
sys.path.insert(0, "/opt/trn_rl_repo")
import numpy as np
import ml_dtypes
from contextlib import ExitStack


def _pin_act_table_set():
    """Reorder act_info.json so natural_log_exp_and_others is the FIRST set:
    exp/identity/copy then all resolve to one table (no mid-kernel
    reloads)."""
    if os.environ.get("BASS_ACT_ROOT_JSON_PATH"):
        return
    try:
        from neuronxcc.driver.Job import Job
        from neuronxcc.driver.jobs.support import FindActInfo
        src = FindActInfo.findActInfoFile(Job.getPackageDir(), "gen3")
    except Exception:
        return
    with open(src) as f:
        info = json.load(f)
    sets = info["act_func_sets"]
    first = [s for s in sets if s["name"] == "natural_log_exp_and_others"]
    if not first:
        return
    rest = [s for s in sets if s["name"] != "natural_log_exp_and_others"]
    d = tempfile.mkdtemp(prefix="act_pin_")
    srcdir = os.path.dirname(src)
    for fn in os.listdir(srcdir):
        dst = os.path.join(d, fn)
        if not os.path.exists(dst):
            os.symlink(os.path.join(srcdir, fn), dst)
    out = os.path.join(d, "act_info.json")
    os.unlink(out)
    with open(out, "w") as f:
        json.dump({"pwp_file_keys": info["pwp_file_keys"],
                   "act_func_sets": first + rest}, f)
    os.environ["BASS_ACT_ROOT_JSON_PATH"] = out
    from neuronxcc.driver.jobs.support import FindActInfo as FAI
    FAI.findActInfoFile = lambda *a, **kw: out


import concourse.bass as bass  # noqa: F401
import concourse.tile as tile
from concourse import bacc, mybir, bass_isa
from concourse.bass_utils import run_bass_kernel_spmd

F32 = mybir.dt.float32
BF16 = mybir.dt.bfloat16
FP8 = mybir.dt.float8e4
AF = mybir.ActivationFunctionType
DR = mybir.MatmulPerfMode.DoubleRow
MUL = mybir.AluOpType.mult
ADD = mybir.AluOpType.add

NUM_HEADS = 16
BASE = 10000.0
N_CORES = 8
C = 2048
D = 128
# fp8 projection weights are pre-scaled by WSCALE on the host (w_qkv values
# ~N(0, 1/45) underflow e4m3 normals); the descale folds into the bias-add
# (q,k) / the v PSUM->SBUF copy, so it costs nothing at runtime
WSCALE = 64.0


def build_nc(B, T, HPC):
    """One core's program: HPC heads, all B batches, full T."""
    assert HPC == 2
    CH = 512                 # t-chunk (query block)
    NCC = C // 128           # c-chunks in the contraction dim
    NTC = T // CH            # t-chunks
    NTT = T // 128           # t-tiles
    NJ = 2 * HPC             # q,k d-tiles (q_h0, q_h1, k_h0, k_h1)
    NV = HPC * D             # v columns per core
    WCOLS = 3 * HPC * D      # packed W columns
    SCALE = float(1.0 / math.sqrt(D))

    _pin_act_table_set()
    nc = bacc.Bacc("TRN2", target_bir_lowering=False, debug=False,
                   enable_asserts=False)
    # bf16 x^T only for the tci==0 chunks (tokens 0..CH): those queries'
    # softmax support is tiny, so fp8 projection noise there would blow the
    # error budget — everything later runs the projections in fp8 DoubleRow
    # (256-long contraction per pass = 2x PE rate)
    xtd = nc.dram_tensor("xt", [B, C, CH], BF16, kind="ExternalInput").ap()
    x8d = nc.dram_tensor("x8", [B, NCC // 2, 128, 2, T], FP8,
                         kind="ExternalInput").ap()
    wqd = nc.dram_tensor("wq", [NCC, 128, WCOLS], BF16, kind="ExternalInput").ap()
    wq8d = nc.dram_tensor("wq8", [NCC // 2, 128, 2, WCOLS], FP8,
                          kind="ExternalInput").ap()
    wod = nc.dram_tensor("wo", [HPC, D, C], BF16, kind="ExternalInput").ap()
    cosd = nc.dram_tensor("cos2", [D, T], BF16, kind="ExternalInput").ap()
    sind = nc.dram_tensor("sin2", [D, T], F32, kind="ExternalInput").ap()
    pimd = nc.dram_tensor("pim", [D, D], BF16, kind="ExternalInput").ap()
    bqkd = nc.dram_tensor("bqk", [128, NJ], F32, kind="ExternalInput").ap()
    onecd = nc.dram_tensor("onec", [128, 1], BF16, kind="ExternalInput").ap()
    onerd = nc.dram_tensor("oner", [1, 128], BF16, kind="ExternalInput").ap()
    outd = nc.dram_tensor("outp", [B, T, C], BF16, kind="ExternalOutput").ap()
    # the final chunk's h1 out-proj partial: its h0 half interleaves into
    # h1's attention, the h1 half lands here and the host adds it in, so the
    # kernel tail only carries half the output DMA
    out2d = nc.dram_tensor("outp2", [CH, C], BF16, kind="ExternalOutput").ap()

    with tile.TileContext(nc) as tc, ExitStack() as ctx, \
            nc.allow_low_precision(reason="bf16 kernel; 2e-2 rel-err budget"):
        consts = ctx.enter_context(tc.tile_pool(name="consts", bufs=1))
        xtp = ctx.enter_context(tc.tile_pool(name="xtp", bufs=36))
        qkp = ctx.enter_context(tc.tile_pool(name="qkp", bufs=2))
        kvp = ctx.enter_context(tc.tile_pool(name="kvp", bufs=2))
        tmp = ctx.enter_context(tc.tile_pool(name="tmp", bufs=2))
        ep = ctx.enter_context(tc.tile_pool(name="ep", bufs=4))
        esp = ctx.enter_context(tc.tile_pool(name="esp", bufs=2))
        rp = ctx.enter_context(tc.tile_pool(name="rp", bufs=2))
        rbp = ctx.enter_context(tc.tile_pool(name="rbp", bufs=2))
        usp = ctx.enter_context(tc.tile_pool(name="usp", bufs=2))
        osp = ctx.enter_context(tc.tile_pool(name="osp", bufs=2))
        pa = ctx.enter_context(tc.tile_pool(name="pa", bufs=2, space="PSUM"))
        sp = ctx.enter_context(tc.tile_pool(name="sp", bufs=3, space="PSUM"))
        up = ctx.enter_context(tc.tile_pool(name="up", bufs=2, space="PSUM"))
        osb = ctx.enter_context(tc.tile_pool(name="osb", bufs=1, space="PSUM"))

        # startup dispatch split: sync (SP) queue carries x + big tables,
        # the idle Activation queue carries small consts + wq — both are
        # hwdge engines, and dma_start dispatch is ~620ns serial per engine.
        # The tiny consts dispatch mid-way through the x/wq interleave (see
        # below): first-use is chunk-0's bias-add at ~20us, and putting them
        # first would delay the first (x,wq) pair by ~2.5us of dispatch.
        bqk_sb = consts.tile([128, NJ], F32)
        pim_sb = consts.tile([128, 128], BF16)
        onec_sb = consts.tile([128, 1], BF16)
        oner_sb = consts.tile([33, 128], BF16)

        def dma_small_consts():
            nc.scalar.dma_start(out=bqk_sb, in_=bqkd)
            nc.scalar.dma_start(out=pim_sb, in_=pimd)
            nc.scalar.dma_start(out=onec_sb, in_=onecd)
            nc.scalar.dma_start(out=oner_sb[0:1, :], in_=onerd)
            nc.scalar.dma_start(out=oner_sb[32:33, :], in_=onerd)
        # per-ci tiles for x and wq: fine-grained dependencies and parallel
        # DMA queues (batched multi-ci DMAs measured slower: single-queue
        # serialization + slower PE streams from 3D tile slices). Only the
        # sync(SP) + scalar(Act) queues are hardware dge; gpsimd's is a
        # software dge (descriptor gen on the Pool ALU) and far slower.
        def dma_x(b, tci):
            """Fetch one chunk's x^T: bf16 per-ci tiles for tci==0, fp8
            ci-pair tiles (DoubleRow layout) otherwise."""
            ts0 = tci * CH
            xts = []
            if tci == 0:
                for ci in range(NCC):
                    xt_t = xtp.tile([128, CH], BF16, tag="xt", name=f"xt{ci}")
                    nc.sync.dma_start(
                        out=xt_t,
                        in_=xtd[b, ci * 128:(ci + 1) * 128, 0:CH])
                    xts.append(xt_t)
            else:
                for g in range(NCC // 2):
                    xt_t = xtp.tile([128, 2, CH], FP8, tag="xt8", bufs=18,
                                    name=f"xt8_{g}")
                    nc.sync.dma_start(
                        out=xt_t, in_=x8d[b, g, :, :, ts0:ts0 + CH])
                    xts.append(xt_t)
            return xts

        # startup, ordered by first-use time under the ~280GB/s per-core HBM
        # cap: (x0, wq) pairs interleaved across both hwdge queues land
        # first (consumed from ~1us by the ci-outer startup proj), then
        # cos+sin (chunk-0 RoPE ~20us), then chunk-1's fp8 x + wq8
        # (~32us), then wo (~55us)
        wq_sb = [consts.tile([128, WCOLS], BF16, name=f"wq{ci}")
                 for ci in range(NCC)]
        xts_next = []
        for ci in range(NCC):
            xt_t = xtp.tile([128, CH], BF16, tag="xt", name=f"xt{ci}")
            nc.sync.dma_start(
                out=xt_t, in_=xtd[0, ci * 128:(ci + 1) * 128, 0:CH])
            xts_next.append(xt_t)
            (nc.scalar if ci % 2 == 0 else nc.sync).dma_start(
                out=wq_sb[ci], in_=wqd[ci])
            if ci == 5:
                dma_small_consts()
        cos_sb = consts.tile([128, T], BF16)
        nc.sync.dma_start(out=cos_sb, in_=cosd)
        sin_sb = consts.tile([128, T], F32)
        nc.scalar.dma_start(out=sin_sb, in_=sind)
        wq8_sb = [consts.tile([128, 2, WCOLS], FP8, name=f"wq8_{g}")
                  for g in range(NCC // 2)]
        for g in range(NCC // 2):
            nc.scalar.dma_start(out=wq8_sb[g], in_=wq8d[g])
        # chunk 1's fp8 x ahead of wo: it's needed ~20us earlier
        xts_pre1 = dma_x(0, 1)
        wo_sb = consts.tile([128, HPC, C], BF16)
        for h in range(HPC):
            nc.sync.dma_start(out=wo_sb[:, h, :], in_=wod[h])

        def outproj_gen(us_t, ts0, b, slots=None, split_dma=False,
                        heads=(0, 1), to_out2=False):
            """One (tt, cc) 512-col out-proj step per next(): 2 accumulating
            matmuls into one PSUM bank + a copy into the bf16 staging tile;
            DMA per tt row-block. Interleaved into attention as PE filler.

            slots: list of (pool, tag) PSUM sources rotated per step so the
            PE never waits on a single bank's drain. The copy out of PSUM is
            split across scalar+vector per half to halve bank-release
            latency. split_dma (final chunk) emits a 512-col DMA per cc,
            alternating the sync and (then-idle) scalar hwdge queues, so
            the 2MB output drain overlaps the out-proj compute instead of
            serializing after it."""
            if slots is None:
                slots = ((osb, "os"),)
            for tt in range(4):
                t0 = ts0 + tt * 128
                ost = osp.tile([128, C], BF16, tag="ost", bufs=4,
                               name=f"ost{tt}")
                for cc in range(4):
                    pool, tag = slots[(4 * tt + cc) % len(slots)]
                    ps = pool.tile([128, 512], F32, tag=tag,
                                   name=f"os{tt}_{cc}")
                    for i, h in enumerate(heads):
                        nc.tensor.matmul(
                            ps, us_t[:, h, tt * 128:(tt + 1) * 128],
                            wo_sb[:, h, cc * 512:(cc + 1) * 512],
                            start=(i == 0), stop=(i == len(heads) - 1))
                    c0 = cc * 512
                    # one full-width copy per step, engines alternating
                    # (gpsimd can't touch PSUM): two independent PE->copy
                    # lanes. Splitting each copy across both engines
                    # instead chains PE->scalar->vector sems serially and
                    # throttles the drain to ~1.1us/step.
                    if (4 * tt + cc) % 2 == 0:
                        nc.scalar.copy(ost[:, c0:c0 + 512], ps)
                    else:
                        nc.vector.tensor_copy(ost[:, c0:c0 + 512], ps)
                    if split_dma:
                        eng = nc.sync if (tt + cc) % 2 == 0 else nc.scalar
                        dst = (out2d[t0 - ts0:t0 - ts0 + 128, c0:c0 + 512]
                               if to_out2
                               else outd[b, t0:t0 + 128, c0:c0 + 512])
                        eng.dma_start(out=dst, in_=ost[:, c0:c0 + 512])
                    yield
                if not split_dma:
                    dst = (out2d[t0 - ts0:t0 - ts0 + 128, :] if to_out2
                           else outd[b, t0:t0 + 128, :])
                    nc.sync.dma_start(out=dst, in_=ost)

        filler = None
        filler_next = [None]
        pfill = [None]
        pending_rb = []

        def nx():
            nonlocal filler
            while True:
                if filler is None and filler_next[0] is not None:
                    filler, filler_next[0] = filler_next[0], None
                if filler is None:
                    break
                try:
                    next(filler)
                    return
                except StopIteration:
                    filler = None
            # out-proj filler dry: fall through to the next chunk's fp8
            # proj stream — always-ready PE work (prefetched x8 + resident
            # wq8), so any stall site still gets covered
            if pfill[0] is not None:
                try:
                    next(pfill[0])
                except StopIteration:
                    pfill[0] = None

        def drain_pfill():
            while pfill[0] is not None:
                try:
                    next(pfill[0])
                except StopIteration:
                    pfill[0] = None

        def drain_rb():
            while pending_rb:
                pending_rb.pop(0)()

        def drain_one():
            if pending_rb:
                pending_rb.pop(0)()

        def outproj_final_gen(us_t, ts0, b):
            """Final chunk's h1 out-proj in two 8-matmul waves across all 8
            PSUM banks (nothing else is live), then copies + per-piece DMAs
            chasing each wave. A per-step matmul->copy->yield drain instead
            throttles to ~840ns/step on PE->copy->PE sem round-trips at the
            DVFS-degraded clock."""
            slots8 = [(osb, "os"), (up, "u"), (up, "u"), (pa, "pa"),
                      (pa, "pa"), (sp, "s"), (sp, "s"), (sp, "s")]
            steps = [(tt, cc) for tt in range(4) for cc in range(4)]
            osts = {}
            for wave in range(2):
                pss = []
                for w in range(8):
                    tt, cc = steps[wave * 8 + w]
                    pool, tag = slots8[w]
                    ps = pool.tile([128, 512], F32, tag=tag,
                                   name=f"fo{tt}_{cc}")
                    nc.tensor.matmul(ps, us_t[:, 1, tt * 128:(tt + 1) * 128],
                                     wo_sb[:, 1, cc * 512:(cc + 1) * 512],
                                     start=True, stop=True)
                    pss.append(ps)
                    yield
                for w in range(8):
                    tt, cc = steps[wave * 8 + w]
                    if tt not in osts:
                        osts[tt] = osp.tile([128, C], BF16, tag="ost",
                                            bufs=4, name=f"fost{tt}")
                    c0 = cc * 512
                    if w % 2 == 0:
                        nc.scalar.copy(osts[tt][:, c0:c0 + 512], pss[w])
                    else:
                        nc.vector.tensor_copy(osts[tt][:, c0:c0 + 512],
                                              pss[w])
                    eng = nc.sync if (tt + cc) % 2 == 0 else nc.scalar
                    eng.dma_start(
                        out=out2d[tt * 128:(tt + 1) * 128, c0:c0 + 512],
                        in_=osts[tt][:, c0:c0 + 512])
                    yield

        def proj8_gen(tci, xts, raws_out, v_dst):
            """The NEXT chunk's fp8 DoubleRow projections, yielded in
            ~2-matmul steps so they interleave into THIS chunk's attention
            as second-priority PE filler (consumed via nx once the
            out-proj filler runs dry; remnant force-drains at the next
            chunk's start, back-to-back pure PE)."""
            for j in range(NJ):
                ps = pa.tile([128, CH], F32, tag="pa", name=f"ppj{j}")
                for g in range(NCC // 2):
                    nc.tensor.matmul(
                        ps, wq8_sb[g][:, :, j * 128:(j + 1) * 128],
                        xts[g], start=(g == 0),
                        stop=(g == NCC // 2 - 1), perf_mode=DR)
                    if g % 2 == 1:
                        yield
                raw = tmp.tile([128, CH], BF16, tag="raw", bufs=8,
                               name=f"praw{j}")
                nc.vector.tensor_scalar(raw, ps, 1.0 / WSCALE,
                                        bqk_sb[:, j:j + 1], op0=MUL, op1=ADD)
                raws_out.append(raw)
            for tt in range(CH // 128):
                ps = pa.tile([128, NV], F32, tag="pa", name=f"ppv{tt}")
                for g in range(NCC // 2):
                    nc.tensor.matmul(
                        ps, xts[g][:, :, tt * 128:(tt + 1) * 128],
                        wq8_sb[g][:, :, NJ * 128:],
                        start=(g == 0), stop=(g == NCC // 2 - 1),
                        perf_mode=DR)
                    if g % 2 == 1:
                        yield
                nc.scalar.activation(
                    v_dst[:, tci * (CH // 128) + tt, :], ps,
                    AF.Identity, scale=1.0 / WSCALE)

        raws_next = []
        for b in range(B):
            k_sb = kvp.tile([128, HPC, T], BF16, tag="k")
            v_sb = kvp.tile([128, NTT, NV], BF16, tag="v")
            for tci in range(NTC):
                ts0 = tci * CH
                # finish the pipelined proj for THIS chunk (emitted during
                # the previous chunk's attention) and take over its raws
                drain_pfill()
                raws_pipe = raws_next[:]
                del raws_next[:]
                # ---- x^T slices: this chunk's were prefetched last chunk;
                # emit the NEXT chunk's DMAs now so they overlap compute
                xts = xts_next
                nb, ntci = (b, tci + 1) if tci + 1 < NTC else (b + 1, 0)
                if b == 0 and tci == 0:
                    xts_next = xts_pre1  # dispatched in the startup section
                else:
                    xts_next = dma_x(nb, ntci) if nb < B else None
                # queue the NEXT chunk's fp8 proj as second-priority filler
                # (same batch only: tci==0 chunks keep their bf16 proj
                # inline)
                if xts_next is not None and nb == b and ntci >= 1:
                    pfill[0] = proj8_gen(ntci, xts_next, raws_next, v_sb)
                if b == 0 and tci == 0:
                    # ---- startup chunk: ci-outer so PE consumes each
                    # (x[ci], wq[ci]) pair as DMA delivers it (8 matmuls =
                    # ~1.3-2.6us per pair vs ~0.65us supply) instead of
                    # stalling through j=0's 16-ci staircase. Uses all 8
                    # PSUM banks — nothing else is live yet.
                    psj = ([pa.tile([128, CH], F32, tag="pa", name=f"pj{j}")
                            for j in range(2)] +
                           [up.tile([128, CH], F32, tag="u", name=f"pj{j}")
                            for j in range(2, 4)])
                    psv = ([sp.tile([128, NV], F32, tag="s", name=f"pv{tt}")
                            for tt in range(3)] +
                           [osb.tile([128, NV], F32, tag="os", name="pv3")])
                    for ci in range(NCC):
                        for j in range(NJ):
                            nc.tensor.matmul(
                                psj[j], wq_sb[ci][:, j * 128:(j + 1) * 128],
                                xts[ci], start=(ci == 0),
                                stop=(ci == NCC - 1))
                        for tt in range(CH // 128):
                            nc.tensor.matmul(
                                psv[tt], xts[ci][:, tt * 128:(tt + 1) * 128],
                                wq_sb[ci][:, NJ * 128:],
                                start=(ci == 0), stop=(ci == NCC - 1))
                    raws = []
                    for j in range(NJ):
                        raw = tmp.tile([128, CH], BF16, tag="raw", bufs=8,
                                       name=f"raw{j}")
                        nc.vector.tensor_scalar_add(raw, psj[j],
                                                    bqk_sb[:, j:j + 1])
                        raws.append(raw)
                    for tt in range(CH // 128):
                        nc.scalar.copy(v_sb[:, tt, :], psv[tt])
                elif tci == 0:
                    # ---- bf16 q,k projection (transposed layout): tokens
                    # 0..CH keep full precision (tiny softmax support)
                    raws = []
                    for j in range(NJ):
                        ps = pa.tile([128, CH], F32, tag="pa", name=f"pj{j}")
                        for ci in range(NCC):
                            nc.tensor.matmul(
                                ps, wq_sb[ci][:, j * 128:(j + 1) * 128],
                                xts[ci], start=(ci == 0),
                                stop=(ci == NCC - 1))
                        raw = tmp.tile([128, CH], BF16, tag="raw", bufs=8,
                                       name=f"raw{j}")
                        nc.vector.tensor_scalar_add(raw, ps, bqk_sb[:, j:j + 1])
                        raws.append(raw)
                        if j <= 1:
                            # previous chunk's deferred h1 l/rb groups, one
                            # per j so this chunk's proj matmuls cover the
                            # Act Ln/Exp latency between them
                            drain_one()
                        if j >= 2:
                            # no fillers in the first ~5us of a chunk: the
                            # previous chunk's us_t may still be in flight
                            nx()
                    # ---- v projection (natural layout)
                    for tt in range(CH // 128):
                        ps = pa.tile([128, NV], F32, tag="pa", name=f"pv{tt}")
                        for ci in range(NCC):
                            nc.tensor.matmul(
                                ps, xts[ci][:, tt * 128:(tt + 1) * 128],
                                wq_sb[ci][:, NJ * 128:],
                                start=(ci == 0), stop=(ci == NCC - 1))
                        nc.scalar.copy(v_sb[:, tci * (CH // 128) + tt, :], ps)
                        nx()
                else:
                    # ---- fp8 projections already streamed through the
                    # previous chunk's attention (proj8_gen); just take the
                    # raws it produced
                    assert len(raws_pipe) == NJ
                    raws = raws_pipe
                # ---- RoPE
                q_t = qkp.tile([128, HPC, CH], BF16, tag="q")
                for j in range(NJ):
                    qp = pa.tile([128, CH], F32, tag="pa", name=f"qp{j}")
                    nc.tensor.matmul(qp, pim_sb, raws[j], start=True, stop=True)
                    t1 = tmp.tile([128, CH], BF16, tag="t1", bufs=4,
                                  name=f"t1_{j}")
                    nc.vector.tensor_mul(t1, raws[j], cos_sb[:, ts0:ts0 + CH])
                    t2 = tmp.tile([128, CH], BF16, tag="t1", bufs=4,
                                  name=f"t2_{j}")
                    nc.vector.tensor_mul(t2, qp, sin_sb[:, ts0:ts0 + CH])
                    dest = (q_t[:, j, :] if j < HPC
                            else k_sb[:, j - HPC, ts0:ts0 + CH])
                    nc.vector.tensor_add(dest, t1, t2)
                    if tci >= 1:
                        # pipelined chunks have no proj-phase drain sites:
                        # run the previous chunk's deferred h1 normalize
                        # here, BEFORE any nx pulls the out-proj filler
                        # that reads its us_t
                        if j == 0:
                            drain_rb()
                        else:
                            nx()
                    else:
                        nx()
                # ---- attention: heads sequential, S pipelined ahead
                us_t = usp.tile([128, HPC, CH], BF16, tag="us")
                lnl = rp.tile([33, CH], F32, tag="lnl", bufs=1)
                r_t = rp.tile([33, CH], BF16, tag="r", bufs=1)
                ns = 4 * tci + 4
                # per-head tails (L matmul / Ln / Exp, then rb broadcast +
                # normalize) are deferred into the NEXT head's S stream /
                # the end-of-chunk filler drain, so their e_sum and scalar
                # latencies are covered by independent PE work
                tail_l = [None]
                tail_rb = [None]
                final = (b == B - 1 and tci == NTC - 1)

                def pop(lst):
                    if lst[0] is not None:
                        fn, lst[0] = lst[0], None
                        fn()
                        return True
                    return False

                def pop_rb(h):
                    if pop(tail_rb) and final and h == 1:
                        # h0's us is written now: queue its half of the
                        # final out-proj to stream through the rest of h1's
                        # attention (after the previous chunk's filler runs
                        # dry) so the kernel tail only carries h1's half.
                        # pa is free here (no successor chunk pipelines into
                        # the final chunk), so give it a second PSUM slot
                        # for the standalone-drain portion
                        filler_next[0] = outproj_gen(
                            us_t, ts0, b, heads=(0,),
                            slots=((osb, "os"), (pa, "pa")))

                h1_tail = [None]
                for h in range(HPC):
                    u_ps = up.tile([128, CH], F32, tag="u", name=f"u{h}")
                    e_sum = esp.tile([128, CH], BF16, tag="es", name=f"es{h}")

                    def flush(ent, h=h, u_ps=u_ps, e_sum=e_sum, ns=ns,
                              tci=tci):
                        si, s_ps, n0 = ent
                        e_t = ep.tile([128, CH], BF16, tag="e",
                                      name=f"e{h}_{si}")
                        nc.scalar.activation(e_t[:, n0:], s_ps[:, n0:], AF.Exp,
                                             scale=SCALE)
                        o = si - 4 * tci
                        if o >= 0:
                            nc.gpsimd.affine_select(
                                out=e_t[:, n0:], in_=e_t[:, n0:],
                                compare_op=mybir.AluOpType.is_ge, fill=0.0,
                                base=0, pattern=[[1, CH - n0]],
                                channel_multiplier=-1)
                        nc.tensor.matmul(u_ps[:, n0:],
                                         v_sb[:, si, h * 128:(h + 1) * 128],
                                         e_t[:, n0:], start=(si == 0),
                                         stop=(si == ns - 1))
                        if si == 0:
                            nc.vector.tensor_copy(e_sum, e_t)
                        else:
                            nc.vector.tensor_add(e_sum[:, n0:], e_sum[:, n0:],
                                                 e_t[:, n0:])

                    pend = []
                    nflush = 0
                    for si in range(ns):
                        o = si - 4 * tci
                        n0 = 128 * o if o > 0 else 0
                        s_ps = sp.tile([128, CH], F32, tag="s",
                                       name=f"s{h}_{si}")
                        nc.tensor.matmul(s_ps[:, n0:],
                                         k_sb[:, h, si * 128:(si + 1) * 128],
                                         q_t[:, h, n0:], start=True, stop=True)
                        pend.append((si, s_ps, n0))
                        if len(pend) >= 3:
                            flush(pend.pop(0))
                            nflush += 1
                            if nflush == 2:
                                pop(tail_l)
                            elif nflush == 5:
                                pop_rb(h)
                            nx()
                    for ent in pend:
                        flush(ent)
                        nflush += 1
                        if nflush == 2:
                            pop(tail_l)
                        elif nflush == 5:
                            pop_rb(h)
                        nx()
                        # the pend-drain flushes emit no new S matmuls, so
                        # the PE has slack here: pull a second filler step
                        # to shrink the end-of-chunk standalone remnant
                        nx()
                    pop(tail_l)
                    pop_rb(h)

                    def l_group(h=h, e_sum=e_sum):
                        l_ps = sp.tile([1, CH], F32, tag="s", name=f"l{h}")
                        nc.tensor.matmul(l_ps, onec_sb, e_sum, start=True,
                                         stop=True)
                        nc.scalar.activation(lnl[32 * h:32 * h + 1, :], l_ps,
                                             AF.Ln)
                        nc.scalar.activation(r_t[32 * h:32 * h + 1, :],
                                             lnl[32 * h:32 * h + 1, :],
                                             AF.Exp, scale=-1.0)

                    def rb_group(h=h, u_ps=u_ps, us_t=us_t, r_t=r_t):
                        rb_ps = sp.tile([128, CH], F32, tag="s",
                                        name=f"rbp{h}")
                        nc.tensor.matmul(rb_ps,
                                         oner_sb[32 * h:32 * h + 1, :],
                                         r_t[32 * h:32 * h + 1, :],
                                         start=True, stop=True)
                        rb_sb = rbp.tile([128, CH], F32, tag="rb",
                                         name=f"rbs{h}")
                        nc.vector.tensor_copy(rb_sb, rb_ps)
                        nc.vector.tensor_mul(us_t[:, h, :], u_ps, rb_sb)

                    if h == 0:
                        tail_l[0] = l_group
                        tail_rb[0] = rb_group
                    else:
                        h1_tail[0] = (l_group, rb_group)
                lg, rg = h1_tail[0]
                if final:
                    # emit h1's L/Ln/Exp before draining the h0 out-proj
                    # remnant: Act computes the reciprocal while the PE
                    # streams filler steps (keeps the DVFS ramp warm), and
                    # rg's rb matmul finds r_t ready right after the drain
                    lg()
                while filler is not None or filler_next[0] is not None:
                    nx()
                if final:
                    rg()
                else:
                    # defer h1's normalize into the next chunk's proj phase:
                    # running it here stalls PE ~0.5-0.8us on the Act
                    # Ln/Exp chain with nothing left to interleave
                    pending_rb.append(lg)
                    pending_rb.append(rg)
                if final:
                    # final drain: only h1's half remains (h0's streamed
                    # during h1's attention into outd); wave layout across
                    # all 8 PSUM banks into the outp2 partial
                    filler = outproj_final_gen(us_t, ts0, b)
                else:
                    filler = outproj_gen(us_t, ts0, b)
        while filler is not None:
            nx()
    nc.compile()
    return nc


def _rope_tables(T):
    half = D // 2
    thetas = BASE ** (-np.arange(half, dtype=np.float32) / half)
    ang = np.arange(T, dtype=np.float32)[:, None] * thetas[None, :]  # (T, half)
    sin = np.sin(ang).astype(np.float32)
    cos = np.cos(ang).astype(np.float32)
    # duplicate per pair along d: table[d, t] = f(t, d//2)
    sin2 = np.repeat(sin.T, 2, axis=0)  # (D, T)
    cos2 = np.repeat(cos.T, 2, axis=0)
    return np.ascontiguousarray(sin2), np.ascontiguousarray(cos2)


def _pi_matrix():
    # qp = PI @ q with qp[2i] = -q[2i+1], qp[2i+1] = q[2i]; matmul takes PI^T
    pim = np.zeros((D, D), dtype=np.float32)
    for i in range(D // 2):
        pim[2 * i + 1, 2 * i] = -1.0
        pim[2 * i, 2 * i + 1] = 1.0
    return pim


_NC_CACHE = {}


def _get_nc(B, T, HPC):
    key = (B, T, HPC)
    if key not in _NC_CACHE:
        _NC_CACHE[key] = build_nc(B, T, HPC)
    return _NC_CACHE[key]


def make_in_maps(x, w_qkv, b_qkv, w_out, n_cores=N_CORES, hpc=None):
    B, T, Cx = x.shape
    assert Cx == C
    HPC = hpc if hpc is not None else NUM_HEADS // n_cores
    CH = 512
    bf = ml_dtypes.bfloat16
    f8 = ml_dtypes.float8_e4m3
    xt_f32 = np.transpose(x, (0, 2, 1))  # (B, C, T) f32
    xt = np.ascontiguousarray(xt_f32[:, :, :CH]).astype(bf)  # chunk-0 bf16
    # fp8 x in DoubleRow ci-pair layout: x8[b, g, p, r, t] = xt[b, (2g+r)*128+p, t]
    x8 = np.ascontiguousarray(
        xt_f32.reshape(B, C // 256, 2, 128, T).transpose(0, 1, 3, 2, 4)
    ).astype(f8)
    sin2, cos2 = _rope_tables(T)
    cos2 = cos2.astype(bf)
    pim = _pi_matrix().astype(bf)
    onec = np.ones((128, 1), dtype=np.float32).astype(bf)
    oner = np.ones((1, 128), dtype=np.float32).astype(bf)
    in_maps = []
    for c in range(n_cores):
        heads = [c * HPC + h for h in range(HPC)]
        rows = np.concatenate(
            [np.arange(h * D, (h + 1) * D) for h in heads] +           # q
            [np.arange(C + h * D, C + (h + 1) * D) for h in heads] +   # k
            [np.arange(2 * C + h * D, 2 * C + (h + 1) * D) for h in heads])  # v
        wq = np.ascontiguousarray(w_qkv[rows].T).reshape(
            C // 128, 128, 3 * HPC * D)
        # fp8 pair layout, prescaled: wq8[g, p, r, :] = WSCALE * wq[2g+r, p, :]
        wq8 = np.ascontiguousarray(
            (WSCALE * wq).reshape(C // 256, 2, 128, 3 * HPC * D)
            .transpose(0, 2, 1, 3)).astype(f8)
        bq = b_qkv[rows[:2 * HPC * D]].reshape(2 * HPC, D).T  # (128, NJ)
        wo = np.stack([np.ascontiguousarray(w_out[:, h * D:(h + 1) * D].T)
                       for h in heads])  # (HPC, D, C)
        in_maps.append({
            "xt": xt,
            "x8": x8,
            "wq": np.ascontiguousarray(wq).astype(bf),
            "wq8": wq8,
            "wo": np.ascontiguousarray(wo).astype(bf),
            "bqk": np.ascontiguousarray(bq, dtype=np.float32),
            "cos2": cos2,
            "sin2": sin2,
            "pim": pim,
            "onec": onec,
            "oner": oner,
        })
    return in_maps


def kernel(x, w_qkv, b_qkv, w_out, b_out):
    x = np.asarray(x, dtype=np.float32)
    w_qkv = np.asarray(w_qkv, dtype=np.float32)
    b_qkv = np.asarray(b_qkv, dtype=np.float32)
    w_out = np.asarray(w_out, dtype=np.float32)
    b_out = np.asarray(b_out, dtype=np.float32)
    B, T, Cx = x.shape
    HPC = NUM_HEADS // N_CORES
    nc = _get_nc(B, T, HPC)

    in_maps = make_in_maps(x, w_qkv, b_qkv, w_out, N_CORES)
    res = run_bass_kernel_spmd(nc, in_maps, core_ids=list(range(N_CORES)))
    out = np.zeros((B, T, C), dtype=np.float64)
    CH = 512
    for c in range(N_CORES):
        out += res.results[c]["outp"].astype(np.float32)
        # final chunk's h1 partial lands in a separate buffer
        out[B - 1, T - CH:T, :] += res.results[c]["outp2"].astype(np.float32)
    b_v = b_qkv[2 * C:]
    const = w_out.astype(np.float64) @ b_v.astype(np.float64) + b_out
    out += const[None, None, :]
    return out.astype(np.float32)



# revision 62
# speedup vs baseline: 1.0320x; 1.0320x over previous
"""Trainium2 Bass kernel v2 for multi-head attention with RoPE (causal).

Contract: kernel(**inputs) takes FULL unsharded inputs
  x (B,T,C) f32, w_qkv (3C,C), b_qkv (3C,), w_out (C,C), b_out (C,)
and returns the FULL (B,T,C) f32 output.

Sharding: heads split across 8 NeuronCores (tensor parallel, 2 heads per
core). Each core computes its heads' attention and a partial output
projection over its 256 columns of att_v; the host sums the 8 partials
(the "all-reduce") and adds the bias constant.

v2 changes vs v1:
- all matmul operands bf16 (halves DMA + SBUF, full PE rate at any tile
  size); PSUM accumulation stays fp32
- softmax denominator via vector-accumulated e_sum + ONE ones-matmul per
  (head, chunk) instead of 320 PE ones-matmuls; 1/L via scalar Ln/Exp and
  a broadcast matmul (gpsimd partition_all_reduce / vector reciprocal
  measured 3.4-3.6us each — far too slow for the critical path)
- output projection interleaved into the attention loop as PE gap filler
- k/v SBUF double-buffered across batches; deeper x-tile prefetch
- output partials written bf16 (halves the write DMA)
"""

import sys, math, os, json, tempfile
sys.path.insert(0, "/opt/trn_rl_repo")
import numpy as np
import ml_dtypes
from contextlib import ExitStack


def _pin_act_table_set():
    """Reorder act_info.json so natural_log_exp_and_others is the FIRST set:
    exp/identity/copy then all resolve to one table (no mid-kernel
    reloads)."""
    if os.environ.get("BASS_ACT_ROOT_JSON_PATH"):
        return
    try:
        from neuronxcc.driver.Job import Job
        from neuronxcc.driver.jobs.support import FindActInfo
        src = FindActInfo.findActInfoFile(Job.getPackageDir(), "gen3")
    except Exception:
        return
    with open(src) as f:
        info = json.load(f)
    sets = info["act_func_sets"]
    first = [s for s in sets if s["name"] == "natural_log_exp_and_others"]
    if not first:
        return
    rest = [s for s in sets if s["name"] != "natural_log_exp_and_others"]
    d = tempfile.mkdtemp(prefix="act_pin_")
    srcdir = os.path.dirname(src)
    for fn in os.listdir(srcdir):
        dst = os.path.join(d, fn)
        if not os.path.exists(dst):
            os.symlink(os.path.join(srcdir, fn), dst)
    out = os.path.join(d, "act_info.json")
    os.unlink(out)
    with open(out, "w") as f:
        json.dump({"pwp_file_keys": info["pwp_file_keys"],
                   "act_func_sets": first + rest}, f)
    os.environ["BASS_ACT_ROOT_JSON_PATH"] = out
    from neuronxcc.driver.jobs.support import FindActInfo as FAI
    FAI.findActInfoFile = lambda *a, **kw: out


import concourse.bass as bass  # noqa: F401
import concourse.tile as tile
from concourse import bacc, mybir, bass_isa
from concourse.bass_utils import run_bass_kernel_spmd

F32 = mybir.dt.float32
BF16 = mybir.dt.bfloat16
FP8 = mybir.dt.float8e4
AF = mybir.ActivationFunctionType
DR = mybir.MatmulPerfMode.DoubleRow
MUL = mybir.AluOpType.mult
ADD = mybir.AluOpType.add

NUM_HEADS = 16
BASE = 10000.0
N_CORES = 8
C = 2048
D = 128
# fp8 projection weights are pre-scaled by WSCALE on the host (w_qkv values
# ~N(0, 1/45) underflow e4m3 normals); the descale folds into the bias-add
# (q,k) / the v PSUM->SBUF copy, so it costs nothing at runtime
WSCALE = 64.0


def build_nc(B, T, HPC):
    """One core's program: HPC heads, all B batches, full T."""
    assert HPC == 2
    CH = 512                 # t-chunk (query block)
    NCC = C // 128           # c-chunks in the contraction dim
    NTC = T // CH            # t-chunks
    NTT = T // 128           # t-tiles
    NJ = 2 * HPC             # q,k d-tiles (q_h0, q_h1, k_h0, k_h1)
    NV = HPC * D             # v columns per core
    WCOLS = 3 * HPC * D      # packed W columns
    SCALE = float(1.0 / math.sqrt(D))

    _pin_act_table_set()
    nc = bacc.Bacc("TRN2", target_bir_lowering=False, debug=False,
                   enable_asserts=False)
    # bf16 x^T only for the tci==0 chunks (tokens 0..CH): those queries'
    # softmax support is tiny, so fp8 projection noise there would blow the
    # error budget — everything later runs the projections in fp8 DoubleRow
    # (256-long contraction per pass = 2x PE rate)
    xtd = nc.dram_tensor("xt", [B, C, CH], BF16, kind="ExternalInput").ap()
    x8d = nc.dram_tensor("x8", [B, NCC // 2, 128, 2, T], FP8,
                         kind="ExternalInput").ap()
    wqd = nc.dram_tensor("wq", [NCC, 128, WCOLS], BF16, kind="ExternalInput").ap()
    wq8d = nc.dram_tensor("wq8", [NCC // 2, 128, 2, WCOLS], FP8,
                          kind="ExternalInput").ap()
    wod = nc.dram_tensor("wo", [HPC, D, C], BF16, kind="ExternalInput").ap()
    cosd = nc.dram_tensor("cos2", [D, T], BF16, kind="ExternalInput").ap()
    sind = nc.dram_tensor("sin2", [D, T], F32, kind="ExternalInput").ap()
    pimd = nc.dram_tensor("pim", [D, D], BF16, kind="ExternalInput").ap()
    bqkd = nc.dram_tensor("bqk", [128, NJ], F32, kind="ExternalInput").ap()
    onecd = nc.dram_tensor("onec", [128, 1], BF16, kind="ExternalInput").ap()
    onerd = nc.dram_tensor("oner", [1, 128], BF16, kind="ExternalInput").ap()
    outd = nc.dram_tensor("outp", [B, T, C], BF16, kind="ExternalOutput").ap()
    # the final chunk's h1 out-proj partial: its h0 half interleaves into
    # h1's attention, the h1 half lands here and the host adds it in, so the
    # kernel tail only carries half the output DMA
    out2d = nc.dram_tensor("outp2", [CH, C], BF16, kind="ExternalOutput").ap()

    with tile.TileContext(nc) as tc, ExitStack() as ctx, \
            nc.allow_low_precision(reason="bf16 kernel; 2e-2 rel-err budget"):
        consts = ctx.enter_context(tc.tile_pool(name="consts", bufs=1))
        xtp = ctx.enter_context(tc.tile_pool(name="xtp", bufs=36))
        qkp = ctx.enter_context(tc.tile_pool(name="qkp", bufs=2))
        kvp = ctx.enter_context(tc.tile_pool(name="kvp", bufs=2))
        tmp = ctx.enter_context(tc.tile_pool(name="tmp", bufs=2))
        ep = ctx.enter_context(tc.tile_pool(name="ep", bufs=4))
        esp = ctx.enter_context(tc.tile_pool(name="esp", bufs=2))
        rp = ctx.enter_context(tc.tile_pool(name="rp", bufs=2))
        rbp = ctx.enter_context(tc.tile_pool(name="rbp", bufs=2))
        usp = ctx.enter_context(tc.tile_pool(name="usp", bufs=2))
        osp = ctx.enter_context(tc.tile_pool(name="osp", bufs=2))
        pa = ctx.enter_context(tc.tile_pool(name="pa", bufs=2, space="PSUM"))
        sp = ctx.enter_context(tc.tile_pool(name="sp", bufs=3, space="PSUM"))
        up = ctx.enter_context(tc.tile_pool(name="up", bufs=2, space="PSUM"))
        osb = ctx.enter_context(tc.tile_pool(name="osb", bufs=1, space="PSUM"))

        # startup dispatch split: sync (SP) queue carries x + big tables,
        # the idle Activation queue carries small consts + wq — both are
        # hwdge engines, and dma_start dispatch is ~620ns serial per engine.
        # The tiny consts dispatch mid-way through the x/wq interleave (see
        # below): first-use is chunk-0's bias-add at ~20us, and putting them
        # first would delay the first (x,wq) pair by ~2.5us of dispatch.
        bqk_sb = consts.tile([128, NJ], F32)
        pim_sb = consts.tile([128, 128], BF16)
        onec_sb = consts.tile([128, 1], BF16)
        oner_sb = consts.tile([33, 128], BF16)

        def dma_small_consts():
            nc.scalar.dma_start(out=bqk_sb, in_=bqkd)
            nc.scalar.dma_start(out=pim_sb, in_=pimd)
            nc.scalar.dma_start(out=onec_sb, in_=onecd)
            nc.scalar.dma_start(out=oner_sb[0:1, :], in_=onerd)
            nc.scalar.dma_start(out=oner_sb[32:33, :], in_=onerd)
        # per-ci tiles for x and wq: fine-grained dependencies and parallel
        # DMA queues (batched multi-ci DMAs measured slower: single-queue
        # serialization + slower PE streams from 3D tile slices). Only the
        # sync(SP) + scalar(Act) queues are hardware dge; gpsimd's is a
        # software dge (descriptor gen on the Pool ALU) and far slower.
        def dma_x(b, tci):
            """Fetch one chunk's x^T: bf16 per-ci tiles for tci==0, fp8
            ci-pair tiles (DoubleRow layout) otherwise."""
            ts0 = tci * CH
            xts = []
            if tci == 0:
                # b>0 chunk-0: only tokens 0..255 keep bf16 projections
                # (emulated on host: moving tokens 256..511 to fp8 leaves
                # the absmax error field unchanged — it's set by later
                # queries); fetch bf16 halves + fp8 pair tiles for the rest
                for ci in range(NCC):
                    xt_t = xtp.tile([128, 256], BF16, tag="xt", name=f"xt{ci}")
                    nc.sync.dma_start(
                        out=xt_t,
                        in_=xtd[b, ci * 128:(ci + 1) * 128, 0:256])
                    xts.append(xt_t)
                for g in range(NCC // 2):
                    xt_t = xtp.tile([128, 2, 256], FP8, tag="xt8b", bufs=9,
                                    name=f"xt8b{g}")
                    nc.sync.dma_start(
                        out=xt_t, in_=x8d[b, g, :, :, 256:512])
                    xts.append(xt_t)
            else:
                for g in range(NCC // 2):
                    xt_t = xtp.tile([128, 2, CH], FP8, tag="xt8", bufs=18,
                                    name=f"xt8_{g}")
                    nc.sync.dma_start(
                        out=xt_t, in_=x8d[b, g, :, :, ts0:ts0 + CH])
                    xts.append(xt_t)
            return xts

        # startup, ordered by first-use time under the ~280GB/s per-core HBM
        # cap: (x0, wq) pairs interleaved across both hwdge queues land
        # first (consumed from ~1us by the ci-outer startup proj), then
        # cos+sin (chunk-0 RoPE ~20us), then chunk-1's fp8 x + wq8
        # (~32us), then wo (~55us)
        wq_sb = [consts.tile([128, WCOLS], BF16, name=f"wq{ci}")
                 for ci in range(NCC)]
        xts_next = []
        for ci in range(NCC):
            xt_t = xtp.tile([128, CH], BF16, tag="xt", name=f"xt{ci}")
            nc.sync.dma_start(
                out=xt_t, in_=xtd[0, ci * 128:(ci + 1) * 128, 0:CH])
            xts_next.append(xt_t)
            (nc.scalar if ci % 2 == 0 else nc.sync).dma_start(
                out=wq_sb[ci], in_=wqd[ci])
            if ci == 5:
                dma_small_consts()
        cos_sb = consts.tile([128, T], BF16)
        nc.sync.dma_start(out=cos_sb, in_=cosd)
        sin_sb = consts.tile([128, T], F32)
        nc.scalar.dma_start(out=sin_sb, in_=sind)
        wq8_sb = [consts.tile([128, 2, WCOLS], FP8, name=f"wq8_{g}")
                  for g in range(NCC // 2)]
        for g in range(NCC // 2):
            nc.scalar.dma_start(out=wq8_sb[g], in_=wq8d[g])
        # chunk 1's fp8 x ahead of wo: it's needed ~20us earlier
        xts_pre1 = dma_x(0, 1)
        wo_sb = consts.tile([128, HPC, C], BF16)
        for h in range(HPC):
            nc.sync.dma_start(out=wo_sb[:, h, :], in_=wod[h])

        def outproj_gen(us_t, ts0, b, slots=None, split_dma=False,
                        heads=(0, 1), to_out2=False):
            """One (tt, cc) 512-col out-proj step per next(): 2 accumulating
            matmuls into one PSUM bank + a copy into the bf16 staging tile;
            DMA per tt row-block. Interleaved into attention as PE filler.

            slots: list of (pool, tag) PSUM sources rotated per step so the
            PE never waits on a single bank's drain. The copy out of PSUM is
            split across scalar+vector per half to halve bank-release
            latency. split_dma (final chunk) emits a 512-col DMA per cc,
            alternating the sync and (then-idle) scalar hwdge queues, so
            the 2MB output drain overlaps the out-proj compute instead of
            serializing after it."""
            if slots is None:
                slots = ((osb, "os"),)
            for tt in range(4):
                t0 = ts0 + tt * 128
                ost = osp.tile([128, C], BF16, tag="ost", bufs=4,
                               name=f"ost{tt}")
                for cc in range(4):
                    pool, tag = slots[(4 * tt + cc) % len(slots)]
                    ps = pool.tile([128, 512], F32, tag=tag,
                                   name=f"os{tt}_{cc}")
                    for i, h in enumerate(heads):
                        nc.tensor.matmul(
                            ps, us_t[:, h, tt * 128:(tt + 1) * 128],
                            wo_sb[:, h, cc * 512:(cc + 1) * 512],
                            start=(i == 0), stop=(i == len(heads) - 1))
                    c0 = cc * 512
                    # one full-width copy per step, engines alternating
                    # (gpsimd can't touch PSUM): two independent PE->copy
                    # lanes. Splitting each copy across both engines
                    # instead chains PE->scalar->vector sems serially and
                    # throttles the drain to ~1.1us/step.
                    if (4 * tt + cc) % 2 == 0:
                        nc.scalar.copy(ost[:, c0:c0 + 512], ps)
                    else:
                        nc.vector.tensor_copy(ost[:, c0:c0 + 512], ps)
                    if split_dma:
                        eng = nc.sync if (tt + cc) % 2 == 0 else nc.scalar
                        dst = (out2d[t0 - ts0:t0 - ts0 + 128, c0:c0 + 512]
                               if to_out2
                               else outd[b, t0:t0 + 128, c0:c0 + 512])
                        eng.dma_start(out=dst, in_=ost[:, c0:c0 + 512])
                    yield
                if not split_dma:
                    dst = (out2d[t0 - ts0:t0 - ts0 + 128, :] if to_out2
                           else outd[b, t0:t0 + 128, :])
                    nc.sync.dma_start(out=dst, in_=ost)

        filler = None
        filler_next = [None]
        pfill = [None]
        pending_rb = []

        def nx():
            nonlocal filler
            while True:
                if filler is None and filler_next[0] is not None:
                    filler, filler_next[0] = filler_next[0], None
                if filler is None:
                    break
                try:
                    next(filler)
                    return
                except StopIteration:
                    filler = None
            # out-proj filler dry: fall through to the next chunk's fp8
            # proj stream — always-ready PE work (prefetched x8 + resident
            # wq8), so any stall site still gets covered
            if pfill[0] is not None:
                try:
                    next(pfill[0])
                except StopIteration:
                    pfill[0] = None

        def drain_pfill():
            while pfill[0] is not None:
                try:
                    next(pfill[0])
                except StopIteration:
                    pfill[0] = None

        def drain_rb():
            while pending_rb:
                pending_rb.pop(0)()

        def drain_one():
            if pending_rb:
                pending_rb.pop(0)()

        def outproj_final_gen(us_t, ts0, b):
            """Final chunk's h1 out-proj in two 8-matmul waves across all 8
            PSUM banks (nothing else is live), then copies + per-piece DMAs
            chasing each wave. A per-step matmul->copy->yield drain instead
            throttles to ~840ns/step on PE->copy->PE sem round-trips at the
            DVFS-degraded clock."""
            slots8 = [(osb, "os"), (up, "u"), (up, "u"), (pa, "pa"),
                      (pa, "pa"), (sp, "s"), (sp, "s"), (sp, "s")]
            steps = [(tt, cc) for tt in range(4) for cc in range(4)]
            osts = {}
            for wave in range(2):
                pss = []
                for w in range(8):
                    tt, cc = steps[wave * 8 + w]
                    pool, tag = slots8[w]
                    ps = pool.tile([128, 512], F32, tag=tag,
                                   name=f"fo{tt}_{cc}")
                    nc.tensor.matmul(ps, us_t[:, 1, tt * 128:(tt + 1) * 128],
                                     wo_sb[:, 1, cc * 512:(cc + 1) * 512],
                                     start=True, stop=True)
                    pss.append(ps)
                    yield
                for w in range(8):
                    tt, cc = steps[wave * 8 + w]
                    if tt not in osts:
                        osts[tt] = osp.tile([128, C], BF16, tag="ost",
                                            bufs=4, name=f"fost{tt}")
                    c0 = cc * 512
                    if w % 2 == 0:
                        nc.scalar.copy(osts[tt][:, c0:c0 + 512], pss[w])
                    else:
                        nc.vector.tensor_copy(osts[tt][:, c0:c0 + 512],
                                              pss[w])
                    eng = nc.sync if (tt + cc) % 2 == 0 else nc.scalar
                    eng.dma_start(
                        out=out2d[tt * 128:(tt + 1) * 128, c0:c0 + 512],
                        in_=osts[tt][:, c0:c0 + 512])
                    yield

        def proj8_gen(tci, xts, raws_out, v_dst):
            """The NEXT chunk's fp8 DoubleRow projections, yielded in
            ~2-matmul steps so they interleave into THIS chunk's attention
            as second-priority PE filler (consumed via nx once the
            out-proj filler runs dry; remnant force-drains at the next
            chunk's start, back-to-back pure PE)."""
            for j in range(NJ):
                ps = pa.tile([128, CH], F32, tag="pa", name=f"ppj{j}")
                for g in range(NCC // 2):
                    nc.tensor.matmul(
                        ps, wq8_sb[g][:, :, j * 128:(j + 1) * 128],
                        xts[g], start=(g == 0),
                        stop=(g == NCC // 2 - 1), perf_mode=DR)
                    if g % 2 == 1:
                        yield
                raw = tmp.tile([128, CH], BF16, tag="raw", bufs=8,
                               name=f"praw{j}")
                nc.vector.tensor_scalar(raw, ps, 1.0 / WSCALE,
                                        bqk_sb[:, j:j + 1], op0=MUL, op1=ADD)
                raws_out.append(raw)
            for tt in range(CH // 128):
                ps = pa.tile([128, NV], F32, tag="pa", name=f"ppv{tt}")
                for g in range(NCC // 2):
                    nc.tensor.matmul(
                        ps, xts[g][:, :, tt * 128:(tt + 1) * 128],
                        wq8_sb[g][:, :, NJ * 128:],
                        start=(g == 0), stop=(g == NCC // 2 - 1),
                        perf_mode=DR)
                    if g % 2 == 1:
                        yield
                nc.scalar.activation(
                    v_dst[:, tci * (CH // 128) + tt, :], ps,
                    AF.Identity, scale=1.0 / WSCALE)

        raws_next = []
        for b in range(B):
            k_sb = kvp.tile([128, HPC, T], BF16, tag="k")
            v_sb = kvp.tile([128, NTT, NV], BF16, tag="v")
            for tci in range(NTC):
                ts0 = tci * CH
                # finish the pipelined proj for THIS chunk (emitted during
                # the previous chunk's attention) and take over its raws
                drain_pfill()
                raws_pipe = raws_next[:]
                del raws_next[:]
                # ---- x^T slices: this chunk's were prefetched last chunk;
                # emit the NEXT chunk's DMAs now so they overlap compute
                xts = xts_next
                nb, ntci = (b, tci + 1) if tci + 1 < NTC else (b + 1, 0)
                if b == 0 and tci == 0:
                    xts_next = xts_pre1  # dispatched in the startup section
                else:
                    xts_next = dma_x(nb, ntci) if nb < B else None
                # queue the NEXT chunk's fp8 proj as second-priority filler
                # (same batch only: tci==0 chunks keep their bf16 proj
                # inline)
                if xts_next is not None and nb == b and ntci >= 1:
                    pfill[0] = proj8_gen(ntci, xts_next, raws_next, v_sb)
                if b == 0 and tci == 0:
                    # ---- startup chunk: ci-outer so PE consumes each
                    # (x[ci], wq[ci]) pair as DMA delivers it (8 matmuls =
                    # ~1.3-2.6us per pair vs ~0.65us supply) instead of
                    # stalling through j=0's 16-ci staircase. Uses all 8
                    # PSUM banks — nothing else is live yet.
                    psj = ([pa.tile([128, CH], F32, tag="pa", name=f"pj{j}")
                            for j in range(2)] +
                           [up.tile([128, CH], F32, tag="u", name=f"pj{j}")
                            for j in range(2, 4)])
                    psv = ([sp.tile([128, NV], F32, tag="s", name=f"pv{tt}")
                            for tt in range(3)] +
                           [osb.tile([128, NV], F32, tag="os", name="pv3")])
                    for ci in range(NCC):
                        for j in range(NJ):
                            nc.tensor.matmul(
                                psj[j], wq_sb[ci][:, j * 128:(j + 1) * 128],
                                xts[ci], start=(ci == 0),
                                stop=(ci == NCC - 1))
                        for tt in range(CH // 128):
                            nc.tensor.matmul(
                                psv[tt], xts[ci][:, tt * 128:(tt + 1) * 128],
                                wq_sb[ci][:, NJ * 128:],
                                start=(ci == 0), stop=(ci == NCC - 1))
                    raws = []
                    for j in range(NJ):
                        raw = tmp.tile([128, CH], BF16, tag="raw", bufs=8,
                                       name=f"raw{j}")
                        nc.vector.tensor_scalar_add(raw, psj[j],
                                                    bqk_sb[:, j:j + 1])
                        raws.append(raw)
                    for tt in range(CH // 128):
                        nc.scalar.copy(v_sb[:, tt, :], psv[tt])
                elif tci == 0:
                    # ---- chunk-0 (b>0) projections, split at token 256:
                    # first half bf16 (small softmax support), second half
                    # fp8 DoubleRow (host-emulated: no absmax-error change)
                    x8ts = xts[NCC:]
                    raws = []
                    for j in range(NJ):
                        psa = pa.tile([128, 256], F32, tag="pa",
                                      name=f"pja{j}")
                        for ci in range(NCC):
                            nc.tensor.matmul(
                                psa, wq_sb[ci][:, j * 128:(j + 1) * 128],
                                xts[ci], start=(ci == 0),
                                stop=(ci == NCC - 1))
                        psb = pa.tile([128, 256], F32, tag="pa",
                                      name=f"pjb{j}")
                        for g in range(NCC // 2):
                            nc.tensor.matmul(
                                psb, wq8_sb[g][:, :, j * 128:(j + 1) * 128],
                                x8ts[g], start=(g == 0),
                                stop=(g == NCC // 2 - 1), perf_mode=DR)
                        raw = tmp.tile([128, CH], BF16, tag="raw", bufs=8,
                                       name=f"raw{j}")
                        nc.vector.tensor_scalar_add(raw[:, 0:256], psa,
                                                    bqk_sb[:, j:j + 1])
                        nc.vector.tensor_scalar(
                            raw[:, 256:512], psb, 1.0 / WSCALE,
                            bqk_sb[:, j:j + 1], op0=MUL, op1=ADD)
                        raws.append(raw)
                        if j <= 1:
                            # previous chunk's deferred h1 l/rb groups, one
                            # per j so this chunk's proj matmuls cover the
                            # Act Ln/Exp latency between them
                            drain_one()
                        if j >= 2:
                            # no fillers in the first ~5us of a chunk: the
                            # previous chunk's us_t may still be in flight
                            nx()
                    # ---- v projection (natural layout), same split
                    for tt in range(CH // 128):
                        ps = pa.tile([128, NV], F32, tag="pa", name=f"pv{tt}")
                        if tt < 2:
                            for ci in range(NCC):
                                nc.tensor.matmul(
                                    ps, xts[ci][:, tt * 128:(tt + 1) * 128],
                                    wq_sb[ci][:, NJ * 128:],
                                    start=(ci == 0), stop=(ci == NCC - 1))
                            nc.scalar.copy(v_sb[:, tt, :], ps)
                        else:
                            for g in range(NCC // 2):
                                nc.tensor.matmul(
                                    ps,
                                    x8ts[g][:, :, (tt - 2) * 128:(tt - 1) * 128],
                                    wq8_sb[g][:, :, NJ * 128:],
                                    start=(g == 0), stop=(g == NCC // 2 - 1),
                                    perf_mode=DR)
                            nc.scalar.activation(
                                v_sb[:, tt, :], ps, AF.Identity,
                                scale=1.0 / WSCALE)
                        nx()
                else:
                    # ---- fp8 projections already streamed through the
                    # previous chunk's attention (proj8_gen); just take the
                    # raws it produced
                    assert len(raws_pipe) == NJ
                    raws = raws_pipe
                # ---- RoPE
                q_t = qkp.tile([128, HPC, CH], BF16, tag="q")
                for j in range(NJ):
                    qp = pa.tile([128, CH], F32, tag="pa", name=f"qp{j}")
                    nc.tensor.matmul(qp, pim_sb, raws[j], start=True, stop=True)
                    t1 = tmp.tile([128, CH], BF16, tag="t1", bufs=4,
                                  name=f"t1_{j}")
                    nc.vector.tensor_mul(t1, raws[j], cos_sb[:, ts0:ts0 + CH])
                    t2 = tmp.tile([128, CH], BF16, tag="t1", bufs=4,
                                  name=f"t2_{j}")
                    nc.vector.tensor_mul(t2, qp, sin_sb[:, ts0:ts0 + CH])
                    dest = (q_t[:, j, :] if j < HPC
                            else k_sb[:, j - HPC, ts0:ts0 + CH])
                    nc.vector.tensor_add(dest, t1, t2)
                    if tci >= 1:
                        # pipelined chunks have no proj-phase drain sites:
                        # run the previous chunk's deferred h1 normalize
                        # here, BEFORE any nx pulls the out-proj filler
                        # that reads its us_t
                        if j == 0:
                            drain_rb()
                        else:
                            nx()
                    else:
                        nx()
                # ---- attention: heads sequential, S pipelined ahead
                us_t = usp.tile([128, HPC, CH], BF16, tag="us")
                lnl = rp.tile([33, CH], F32, tag="lnl", bufs=1)
                r_t = rp.tile([33, CH], BF16, tag="r", bufs=1)
                ns = 4 * tci + 4
                # per-head tails (L matmul / Ln / Exp, then rb broadcast +
                # normalize) are deferred into the NEXT head's S stream /
                # the end-of-chunk filler drain, so their e_sum and scalar
                # latencies are covered by independent PE work
                tail_l = [None]
                tail_rb = [None]
                final = (b == B - 1 and tci == NTC - 1)

                def pop(lst):
                    if lst[0] is not None:
                        fn, lst[0] = lst[0], None
                        fn()
                        return True
                    return False

                def pop_rb(h):
                    if pop(tail_rb) and final and h == 1:
                        # h0's us is written now: queue its half of the
                        # final out-proj to stream through the rest of h1's
                        # attention (after the previous chunk's filler runs
                        # dry) so the kernel tail only carries h1's half.
                        # pa is free here (no successor chunk pipelines into
                        # the final chunk), so give it a second PSUM slot
                        # for the standalone-drain portion
                        filler_next[0] = outproj_gen(
                            us_t, ts0, b, heads=(0,),
                            slots=((osb, "os"), (pa, "pa")))

                h1_tail = [None]
                for h in range(HPC):
                    u_ps = up.tile([128, CH], F32, tag="u", name=f"u{h}")
                    e_sum = esp.tile([128, CH], BF16, tag="es", name=f"es{h}")

                    def flush(ent, h=h, u_ps=u_ps, e_sum=e_sum, ns=ns,
                              tci=tci):
                        si, s_ps, n0 = ent
                        e_t = ep.tile([128, CH], BF16, tag="e",
                                      name=f"e{h}_{si}")
                        nc.scalar.activation(e_t[:, n0:], s_ps[:, n0:], AF.Exp,
                                             scale=SCALE)
                        o = si - 4 * tci
                        if o >= 0:
                            nc.gpsimd.affine_select(
                                out=e_t[:, n0:], in_=e_t[:, n0:],
                                compare_op=mybir.AluOpType.is_ge, fill=0.0,
                                base=0, pattern=[[1, CH - n0]],
                                channel_multiplier=-1)
                        nc.tensor.matmul(u_ps[:, n0:],
                                         v_sb[:, si, h * 128:(h + 1) * 128],
                                         e_t[:, n0:], start=(si == 0),
                                         stop=(si == ns - 1))
                        if si == 0:
                            nc.vector.tensor_copy(e_sum, e_t)
                        else:
                            nc.vector.tensor_add(e_sum[:, n0:], e_sum[:, n0:],
                                                 e_t[:, n0:])

                    pend = []
                    nflush = 0
                    for si in range(ns):
                        o = si - 4 * tci
                        n0 = 128 * o if o > 0 else 0
                        s_ps = sp.tile([128, CH], F32, tag="s",
                                       name=f"s{h}_{si}")
                        nc.tensor.matmul(s_ps[:, n0:],
                                         k_sb[:, h, si * 128:(si + 1) * 128],
                                         q_t[:, h, n0:], start=True, stop=True)
                        pend.append((si, s_ps, n0))
                        if len(pend) >= 3:
                            flush(pend.pop(0))
                            nflush += 1
                            if nflush == 2:
                                pop(tail_l)
                            elif nflush == 5:
                                pop_rb(h)
                            nx()
                    for ent in pend:
                        flush(ent)
                        nflush += 1
                        if nflush == 2:
                            pop(tail_l)
                        elif nflush == 5:
                            pop_rb(h)
                        nx()
                    pop(tail_l)
                    pop_rb(h)

                    def l_group(h=h, e_sum=e_sum):
                        l_ps = sp.tile([1, CH], F32, tag="s", name=f"l{h}")
                        nc.tensor.matmul(l_ps, onec_sb, e_sum, start=True,
                                         stop=True)
                        nc.scalar.activation(lnl[32 * h:32 * h + 1, :], l_ps,
                                             AF.Ln)
                        nc.scalar.activation(r_t[32 * h:32 * h + 1, :],
                                             lnl[32 * h:32 * h + 1, :],
                                             AF.Exp, scale=-1.0)

                    def rb_group(h=h, u_ps=u_ps, us_t=us_t, r_t=r_t):
                        rb_ps = sp.tile([128, CH], F32, tag="s",
                                        name=f"rbp{h}")
                        nc.tensor.matmul(rb_ps,
                                         oner_sb[32 * h:32 * h + 1, :],
                                         r_t[32 * h:32 * h + 1, :],
                                         start=True, stop=True)
                        rb_sb = rbp.tile([128, CH], F32, tag="rb",
                                         name=f"rbs{h}")
                        nc.vector.tensor_copy(rb_sb, rb_ps)
                        nc.vector.tensor_mul(us_t[:, h, :], u_ps, rb_sb)

                    if h == 0:
                        tail_l[0] = l_group
                        tail_rb[0] = rb_group
                    else:
                        h1_tail[0] = (l_group, rb_group)
                lg, rg = h1_tail[0]
                if final:
                    # emit h1's L/Ln/Exp before draining the h0 out-proj
                    # remnant: Act computes the reciprocal while the PE
                    # streams filler steps (keeps the DVFS ramp warm), and
                    # rg's rb matmul finds r_t ready right after the drain
                    lg()
                while filler is not None or filler_next[0] is not None:
                    nx()
                if final:
                    rg()
                else:
                    # defer h1's normalize into the next chunk's proj phase:
                    # running it here stalls PE ~0.5-0.8us on the Act
                    # Ln/Exp chain with nothing left to interleave
                    pending_rb.append(lg)
                    pending_rb.append(rg)
                if final:
                    # final drain: only h1's half remains (h0's streamed
                    # during h1's attention into outd); wave layout across
                    # all 8 PSUM banks into the outp2 partial
                    filler = outproj_final_gen(us_t, ts0, b)
                else:
                    filler = outproj_gen(us_t, ts0, b)
        while filler is not None:
            nx()
    nc.compile()
    return nc


def _rope_tables(T):
    half = D // 2
    thetas = BASE ** (-np.arange(half, dtype=np.float32) / half)
    ang = np.arange(T, dtype=np.float32)[:, None] * thetas[None, :]  # (T, half)
    sin = np.sin(ang).astype(np.float32)
    cos = np.cos(ang).astype(np.float32)
    # duplicate per pair along d: table[d, t] = f(t, d//2)
    sin2 = np.repeat(sin.T, 2, axis=0)  # (D, T)
    cos2 = np.repeat(cos.T, 2, axis=0)
    return np.ascontiguousarray(sin2), np.ascontiguousarray(cos2)


def _pi_matrix():
    # qp = PI @ q with qp[2i] = -q[2i+1], qp[2i+1] = q[2i]; matmul takes PI^T
    pim = np.zeros((D, D), dtype=np.float32)
    for i in range(D // 2):
        pim[2 * i + 1, 2 * i] = -1.0
        pim[2 * i, 2 * i + 1] = 1.0
    return pim


_NC_CACHE = {}


def _get_nc(B, T, HPC):
    key = (B, T, HPC)
    if key not in _NC_CACHE:
        _NC_CACHE[key] = build_nc(B, T, HPC)
    return _NC_CACHE[key]


def make_in_maps(x, w_qkv, b_qkv, w_out, n_cores=N_CORES, hpc=None):
    B, T, Cx = x.shape
    assert Cx == C
    HPC = hpc if hpc is not None else NUM_HEADS // n_cores
    CH = 512
    bf = ml_dtypes.bfloat16
    f8 = ml_dtypes.float8_e4m3
    xt_f32 = np.transpose(x, (0, 2, 1))  # (B, C, T) f32
    xt = np.ascontiguousarray(xt_f32[:, :, :CH]).astype(bf)  # chunk-0 bf16
    # fp8 x in DoubleRow ci-pair layout: x8[b, g, p, r, t] = xt[b, (2g+r)*128+p, t]
    x8 = np.ascontiguousarray(
        xt_f32.reshape(B, C // 256, 2, 128, T).transpose(0, 1, 3, 2, 4)
    ).astype(f8)
    sin2, cos2 = _rope_tables(T)
    cos2 = cos2.astype(bf)
    pim = _pi_matrix().astype(bf)
    onec = np.ones((128, 1), dtype=np.float32).astype(bf)
    oner = np.ones((1, 128), dtype=np.float32).astype(bf)
    in_maps = []
    for c in range(n_cores):
        heads = [c * HPC + h for h in range(HPC)]
        rows = np.concatenate(
            [np.arange(h * D, (h + 1) * D) for h in heads] +           # q
            [np.arange(C + h * D, C + (h + 1) * D) for h in heads] +   # k
            [np.arange(2 * C + h * D, 2 * C + (h + 1) * D) for h in heads])  # v
        wq = np.ascontiguousarray(w_qkv[rows].T).reshape(
            C // 128, 128, 3 * HPC * D)
        # fp8 pair layout, prescaled: wq8[g, p, r, :] = WSCALE * wq[2g+r, p, :]
        wq8 = np.ascontiguousarray(
            (WSCALE * wq).reshape(C // 256, 2, 128, 3 * HPC * D)
            .transpose(0, 2, 1, 3)).astype(f8)
        bq = b_qkv[rows[:2 * HPC * D]].reshape(2 * HPC, D).T  # (128, NJ)
        wo = np.stack([np.ascontiguousarray(w_out[:, h * D:(h + 1) * D].T)
                       for h in heads])  # (HPC, D, C)
        in_maps.append({
            "xt": xt,
            "x8": x8,
            "wq": np.ascontiguousarray(wq).astype(bf),
            "wq8": wq8,
            "wo": np.ascontiguousarray(wo).astype(bf),
            "bqk": np.ascontiguousarray(bq, dtype=np.float32),
            "cos2": cos2,
            "sin2": sin2,
            "pim": pim,
            "onec": onec,
            "oner": oner,
        })
    return in_maps


def kernel(x, w_qkv, b_qkv, w_out, b_out):
    x = np.asarray(x, dtype=np.float32)
    w_qkv = np.asarray(w_qkv, dtype=np.float32)
    b_qkv = np.asarray(b_qkv, dtype=np.float32)
    w_out = np.asarray(w_out, dtype=np.float32)
    b_out = np.asarray(b_out, dtype=np.float32)
    B, T, Cx = x.shape
    HPC = NUM_HEADS // N_CORES
    nc = _get_nc(B, T, HPC)

    in_maps = make_in_maps(x, w_qkv, b_qkv, w_out, N_CORES)
    res = run_bass_kernel_spmd(nc, in_maps, core_ids=list(range(N_CORES)))
    out = np.zeros((B, T, C), dtype=np.float64)
    CH = 512
    for c in range(N_CORES):
        out += res.results[c]["outp"].astype(np.float32)
        # final chunk's h1 partial lands in a separate buffer
        out[B - 1, T - CH:T, :] += res.results[c]["outp2"].astype(np.float32)
    b_v = b_qkv[2 * C:]
    const = w_out.astype(np.float64) @ b_v.astype(np.float64) + b_out
    out += const[None, None, :]
    return out.astype(np.float32)

